# revision 45
# baseline (speedup 1.0000x reference)
"""Trainium2 Bass kernel for nn_Block_58609123721506 (DeltaNet-style block).

Sharding: 8 cores = 2 batches x 4 head-groups (4 heads each).
Per core: bf16 qkv/beta projections, fp16 chunked delta-rule scan
(C=128 chunks, blocked-32 forward substitution with 3-stage Neumann
doubling for the diagonal 32x32 inverses), per-head RMSNorm, AllToAll
to token-shard the output projection, bf16 out-projection, fp32
residual + LayerNorm. Each core returns a disjoint (512, 1024) slice.
"""
import numpy as np

B, T, D, H = 2, 2048, 1024, 16
Dh = 64          # head dim
C = 128          # chunk length
NCH = 4          # heads per core
NCHUNK = T // C  # 16
NCORE = 8
WQKV_COLS = 3 * NCH * Dh + NCH  # 772
SCR_COLS = 3 * NCH * Dh        # 768 : q(256) k(256) kbneg(256)


def _build_program():
    from contextlib import ExitStack
    import concourse.bass as bass
    import concourse.mybir as mybir
    from concourse import bacc
    from concourse.tile import TileContext

    dt = mybir.dt
    F32, F16, BF16 = dt.float32, dt.float16, dt.bfloat16
    AF = mybir.ActivationFunctionType
    ALU = mybir.AluOpType

    nc = bacc.Bacc(None, target_bir_lowering=False, debug=True)

    x_b = nc.declare_dram_parameter("x_b", [T, D], F32, isOutput=False)
    x_res = nc.declare_dram_parameter("x_res", [T // 4, D], F32, isOutput=False)
    wqkv = nc.declare_dram_parameter("wqkv", [D, WQKV_COLS], F32, isOutput=False)
    wo = nc.declare_dram_parameter("wo", [D, D], F32, isOutput=False)
    ogam = nc.declare_dram_parameter("ogam", [NCH, Dh], F32, isOutput=False)
    lng = nc.declare_dram_parameter("lng", [1, D], F32, isOutput=False)
    lnb = nc.declare_dram_parameter("lnb", [1, D], F32, isOutput=False)
    slotmask = nc.declare_dram_parameter("slotmask", [1, 4], F32, isOutput=False)
    m_bdsl = nc.declare_dram_parameter("m_bdsl", [C, C], F16, isOutput=False)
    m_bdsu = nc.declare_dram_parameter("m_bdsu", [C, C], F16, isOutput=False)
    m_su = nc.declare_dram_parameter("m_su", [C, C], F16, isOutput=False)
    m_tiu = nc.declare_dram_parameter("m_tiu", [C, C], F16, isOutput=False)
    ident = nc.declare_dram_parameter("ident", [C, C], F16, isOutput=False)
    out = nc.declare_dram_parameter("out", [T // 4, D], F32, isOutput=True)

    s_xb = nc.dram_tensor("s_xb", [T, D], BF16)
    s_qkb = nc.dram_tensor("s_qkb", [T, SCR_COLS], F16)
    s_pin = nc.dram_tensor("s_pin", [T, D], BF16)
    s_prs = nc.dram_tensor("s_prs", [T // 4, D], BF16)

    def bcast_from_dram(ap_row, parts):
        """AP reading one DRAM row broadcast over `parts` partitions."""
        return bass.AP(tensor=ap_row.tensor, offset=ap_row.offset,
                       ap=[[0, parts]] + ap_row.ap[1:])

    with TileContext(nc) as tc, ExitStack() as ctx:
        const = ctx.enter_context(tc.tile_pool(name="const", bufs=1))
        persist = ctx.enter_context(tc.tile_pool(name="persist", bufs=1))

        # ---- constants ----
        sb_masks = {}
        for name, ap in (("m_bdsl", m_bdsl), ("m_bdsu", m_bdsu),
                         ("m_su", m_su), ("m_tiu", m_tiu), ("ident", ident)):
            t = const.tile([C, C], F16, tag=name)
            nc.sync.dma_start(out=t, in_=ap[:, :])
            sb_masks[name] = t
        # broadcast-materialized row vectors
        sb_gamM = const.tile([128, NCH * Dh], F32, tag="gamM")
        for h in range(NCH):
            nc.gpsimd.dma_start(out=sb_gamM[:, h * Dh:(h + 1) * Dh],
                                in_=bcast_from_dram(ogam[h:h + 1, :], 128))
        sb_lngM = const.tile([128, D], F32, tag="lngM")
        nc.gpsimd.dma_start(out=sb_lngM, in_=bcast_from_dram(lng[0:1, :], 128))
        sb_lnbM = const.tile([128, D], F32, tag="lnbM")
        nc.gpsimd.dma_start(out=sb_lnbM, in_=bcast_from_dram(lnb[0:1, :], 128))
        sb_smask = const.tile([128, 4], F32, tag="smask")
        nc.gpsimd.dma_start(out=sb_smask, in_=bcast_from_dram(slotmask[0:1, :], 128))
        e6 = const.tile([128, 1], F32, tag="e6")
        nc.vector.memset(e6, 1e-6)
        e5 = const.tile([128, 1], F32, tag="e5")
        nc.vector.memset(e5, 1e-5)

        # ---- weights: load fp32, cast to bf16 ----
        sb_wqkv = []
        sb_wo = []
        with tc.tile_pool(name="wstage", bufs=2) as wst:
            for dk in range(8):
                wtmp = wst.tile([128, WQKV_COLS], F32, tag="wstage")
                nc.sync.dma_start(out=wtmp, in_=wqkv[dk * 128:(dk + 1) * 128, :])
                wb = const.tile([128, WQKV_COLS], BF16, tag=f"wqkv{dk}")
                nc.scalar.copy(out=wb, in_=wtmp)
                sb_wqkv.append(wb)
            for dk in range(8):
                wtmp = wst.tile([128, D], F32, tag="wstage2")
                nc.sync.dma_start(out=wtmp, in_=wo[dk * 128:(dk + 1) * 128, :])
                wb = const.tile([128, D], BF16, tag=f"wo{dk}")
                nc.scalar.copy(out=wb, in_=wtmp)
                sb_wo.append(wb)

        sb_k16 = []   # token-layout k fp16 (128, 256) per ti
        sb_vb = []    # beta*v fp16 (128, 256) per ti
        sb_bcol = []  # beta fp32 (128, 4) per ti

        with tc.tile_pool(name="xbt", bufs=1) as xbt_pool:
            # ---- phase 1: x -> bf16 -> DRAM -> transposed tiles ----
            with tc.tile_pool(name="xstage", bufs=2) as xst:
                for ti in range(16):
                    xt = xst.tile([128, D], F32, tag="xload")
                    nc.sync.dma_start(out=xt, in_=x_b[ti * 128:(ti + 1) * 128, :])
                    xc = xst.tile([128, D], BF16, tag="xcast")
                    nc.scalar.copy(out=xc, in_=xt)
                    nc.sync.dma_start(out=s_xb[ti * 128:(ti + 1) * 128, :], in_=xc)
            sb_xbT = []
            for dk in range(8):
                t = xbt_pool.tile([128, T], BF16, tag=f"xbT{dk}")
                nc.sync.dma_start_transpose(out=t, in_=s_xb[:, dk * 128:(dk + 1) * 128])
                sb_xbT.append(t)

            # ---- phase 2: projections (two-pass, sqrt batched once) ----
            sb_act = []    # silu(q|k) f32 per ti
            sb_nb = []     # -beta per ti
            ssqA = persist.tile([128, 16, 8], F32, tag="ssqA")
            with tc.tile_pool(name="proj", bufs=2) as pst, \
                 tc.tile_pool(name="pp1", bufs=2, space="PSUM") as pp1, \
                 tc.tile_pool(name="pp2", bufs=2, space="PSUM") as pp2:
                for ti in range(16):
                    ts_ = slice(ti * 128, (ti + 1) * 128)
                    ps1 = pp1.tile([128, 512], F32, tag="ps1")
                    ps2 = pp2.tile([128, 260], F32, tag="ps2")
                    for dk in range(8):
                        lhs = sb_xbT[dk][:, ts_]
                        nc.tensor.matmul(ps1, lhs, sb_wqkv[dk][:, 0:512],
                                         start=(dk == 0), stop=(dk == 7))
                        nc.tensor.matmul(ps2, lhs, sb_wqkv[dk][:, 512:772],
                                         start=(dk == 0), stop=(dk == 7))
                    # beta = 2*sigmoid(x) = 1 + tanh(x/2); avoids a Sigmoid
                    # table load (Tanh shares the Silu act table)
                    tb = pst.tile([128, NCH], F32, tag="tb")
                    nc.scalar.activation(tb, ps2[:, 256:260], AF.Tanh, scale=0.5)
                    bcol = persist.tile([128, NCH], F32, tag=f"bcol{ti}")
                    nc.vector.tensor_scalar_add(bcol, tb, 1.0)
                    nbcol = persist.tile([128, NCH], F32, tag=f"nbcol{ti}")
                    nc.vector.tensor_scalar(nbcol, tb, 1.0, -1.0,
                                            op0=ALU.add, op1=ALU.mult)
                    sb_bcol.append(bcol)
                    sb_nb.append(nbcol)

                    vb = persist.tile([128, 256], F16, tag=f"vb{ti}")
                    act = persist.tile([128, 512], F32, tag=f"act{ti}")
                    nc.scalar.activation(act, ps1, AF.Silu)
                    sb_act.append(act)
                    sq = pst.tile([128, 512], F32, tag="sq")
                    nc.vector.tensor_mul(sq, act, act)
                    nc.vector.tensor_reduce(
                        ssqA[:, ti, :], bass.AP(tensor=sq.tensor, offset=sq.offset,
                                                ap=sq.ap[:1] + [[64, 8], [1, 64]]),
                        axis=mybir.AxisListType.X, op=ALU.add)
                    for h in range(NCH):
                        hs = slice(h * Dh, (h + 1) * Dh)
                        nc.vector.tensor_scalar_mul(vb[:, hs], ps2[:, hs],
                                                    bcol[:, h:h + 1])
                    sb_vb.append(vb)

                # one table switch (Silu -> Sqrt) for the whole kernel
                nrmA = persist.tile([128, 16, 8], F32, tag="nrmA")
                nc.scalar.activation(nrmA[:, :, :], ssqA[:, :, :], AF.Sqrt, bias=e6)
                rnA = persist.tile([128, 16, 8], F32, tag="rnA")
                nc.vector.reciprocal(rnA[:, :, :], nrmA[:, :, :])

                for ti in range(16):
                    ts_ = slice(ti * 128, (ti + 1) * 128)
                    act = sb_act[ti]
                    k16 = persist.tile([128, 256], F16, tag=f"k16_{ti}")
                    q16 = pst.tile([128, 256], F16, tag="q16")
                    kbn = pst.tile([128, 256], F16, tag="kbn")
                    for h in range(NCH):
                        hs = slice(h * Dh, (h + 1) * Dh)
                        ks_ = slice(256 + h * Dh, 256 + (h + 1) * Dh)
                        nc.vector.tensor_scalar_mul(q16[:, hs], act[:, hs],
                                                    rnA[:, ti, h:h + 1])
                        nc.vector.tensor_scalar_mul(k16[:, hs], act[:, ks_],
                                                    rnA[:, ti, 4 + h:5 + h])
                        nc.vector.tensor_scalar(kbn[:, hs], act[:, ks_],
                                                rnA[:, ti, 4 + h:5 + h],
                                                sb_nb[ti][:, h:h + 1],
                                                op0=ALU.mult, op1=ALU.mult)
                    sb_k16.append(k16)

                    nc.sync.dma_start(out=s_qkb[ts_, 0:256], in_=q16)
                    nc.sync.dma_start(out=s_qkb[ts_, 256:512], in_=k16)
                    nc.sync.dma_start(out=s_qkb[ts_, 512:768], in_=kbn)

        # ---- phase 3+4: chunked delta-rule scan ----
        sb_S32 = []
        sb_S16 = []
        for p in range(2):  # head pairs (0,1) and (2,3)
            s32 = persist.tile([128, Dh], F32, tag=f"S32_{p}")
            s16 = persist.tile([128, Dh], F16, tag=f"S16_{p}")
            nc.vector.memset(s32, 0.0)
            nc.vector.memset(s16, 0.0)
            sb_S32.append(s32)
            sb_S16.append(s16)

        MB_SL = sb_masks["m_bdsl"]
        MB_SU = sb_masks["m_bdsu"]
        M_SU = sb_masks["m_su"]
        M_TIU = sb_masks["m_tiu"]
        IDT = sb_masks["ident"]

        with tc.tile_pool(name="tpool", bufs=4) as tpool, \
             tc.tile_pool(name="dpool", bufs=3) as dpool, \
             tc.tile_pool(name="pg", bufs=2, space="PSUM") as pg, \
             tc.tile_pool(name="pd", bufs=2, space="PSUM") as pd, \
             tc.tile_pool(name="psm", bufs=4, space="PSUM") as psm:
            for ci in range(NCHUNK):
                rs = slice(ci * C, (ci + 1) * C)
                bqT = [tpool.tile([128, 2 * C], F16, tag=f"bqT{p}", name=f"bqT{p}")
                       for p in range(2)]
                kT = [tpool.tile([128, C], F16, tag=f"kT{p}", name=f"kT{p}")
                      for p in range(2)]
                for p in range(2):
                    co = p * 128
                    nc.sync.dma_start_transpose(out=bqT[p][:, 0:C],
                                                in_=s_qkb[rs, 512 + co:512 + co + 128])
                    nc.sync.dma_start_transpose(out=bqT[p][:, C:2 * C],
                                                in_=s_qkb[rs, co:co + 128])
                    nc.sync.dma_start_transpose(out=kT[p],
                                                in_=s_qkb[rs, 256 + co:256 + co + 128])

                onorm = tpool.tile([128, NCH * Dh], BF16, tag="onorm")

                for h in range(NCH):
                    p, sub = h // 2, h % 2
                    po = sub * Dh
                    kTh = kT[p][po:po + Dh, :]
                    qTh = bqT[p][po:po + Dh, C:2 * C]
                    bTh = bqT[p][po:po + Dh, 0:C]
                    S16h = sb_S16[p][po:po + Dh, :]

                    # --- chunk-local prep ---
                    # gneg[i,j] = -beta_j k_i.k_j ; gneg2[i,j] = -beta_i k_i.k_j
                    gall = pg.tile([C, 3 * C], F32, tag="g0")
                    nc.tensor.matmul(gall[:, 0:2 * C], kTh, bqT[p][po:po + Dh, :],
                                     start=True, stop=True, tile_position=(po, 0))
                    gneg = gall[:, 0:C]
                    kq = gall[:, C:2 * C]
                    gneg2 = gall[:, 2 * C:3 * C]
                    nc.tensor.matmul(gneg2, bTh, kTh, start=True, stop=True,
                                     tile_position=(po, 0))
                    sqk = dpool.tile([C, C], F16, tag="sqk")
                    nc.vector.tensor_mul(sqk, kq, M_TIU)
                    bdN = dpool.tile([C, C], F16, tag="bdN")
                    nc.vector.tensor_mul(bdN, gneg2, MB_SL)
                    ntil = dpool.tile([C, C], F16, tag="ntil")
                    nc.vector.tensor_mul(ntil, gneg, M_SU)
                    bdNt = dpool.tile([C, C], F16, tag="bdNt")
                    nc.vector.tensor_mul(bdNt, gneg, MB_SU)
                    pt = dpool.tile([C, C], F16, tag="pt")
                    nc.gpsimd.tensor_add(pt, IDT, bdNt)

                    # --- doubling: 3 stages ---
                    bm, bt = bdN, bdNt
                    for j in range(3):
                        bmp = pd.tile([C, C], F32, tag="pdb")
                        nc.tensor.matmul(bmp, bt, bm, start=True, stop=True)
                        bmn = dpool.tile([C, C], F16, tag=f"bm{j}")
                        nc.vector.tensor_copy(bmn, bmp)
                        if j < 2:
                            btp = pd.tile([C, C], F32, tag="pdb")
                            nc.tensor.matmul(btp, bm, bt, start=True, stop=True)
                            btn = dpool.tile([C, C], F16, tag=f"bt{j}")
                            nc.vector.tensor_copy(btn, btp)
                        else:
                            btn = None
                        ptp = pd.tile([C, C], F32, tag="pdb")
                        nc.tensor.matmul(ptp, bmn, pt, start=True, stop=True)
                        ptn = dpool.tile([C, C], F16, tag=f"pt{j}")
                        nc.vector.tensor_add(ptn, pt, ptp)
                        bm, bt, pt = bmn, btn, ptn

                    # --- sequential substitution ---
                    yp = psm.tile([C, Dh], F32, tag="psmall")
                    nc.tensor.matmul(yp, bTh, S16h, start=True, stop=True,
                                     tile_position=(po, 0))
                    rr = tpool.tile([C, Dh], F16, tag="rr")
                    nc.vector.tensor_add(rr, sb_vb[ci][:, h * Dh:(h + 1) * Dh], yp)

                    xv = tpool.tile([C, Dh], F16, tag="xv")
                    xp = psm.tile([C, Dh], F32, tag="psmall")
                    ac = psm.tile([C, Dh], F32, tag="psmall")
                    tst = tpool.tile([C, Dh], F16, tag="tst")
                    for i in range(4):
                        bs = slice(32 * i, 32 * i + 32)
                        if i == 0:
                            nc.tensor.matmul(xp[0:32, :], pt[0:32, 0:32], rr[0:32, :],
                                             start=True, stop=True,
                                             tile_position=(0, 0))
                        else:
                            nc.tensor.matmul(ac[bs, :], ntil[0:32 * i, bs],
                                             xv[0:32 * i, :], start=True, stop=True,
                                             tile_position=(0, 32 * i))
                            nc.vector.tensor_add(tst[bs, :], rr[bs, :], ac[bs, :])
                            nc.tensor.matmul(xp[bs, :], pt[bs, bs], tst[bs, :],
                                             start=True, stop=True,
                                             tile_position=(32 * i, 32 * i))
                        nc.scalar.copy(out=xv[bs, :], in_=xp[bs, :])

                    # --- output ---
                    op_ = psm.tile([C, Dh], F32, tag="psmall")
                    nc.tensor.matmul(op_, qTh, S16h, start=True, stop=False,
                                     tile_position=(po, 0))
                    nc.tensor.matmul(op_, sqk, xv, start=False, stop=True)
                    osq = tpool.tile([C, Dh], F32, tag="osq")
                    ssq = tpool.tile([C, 1], F32, tag="ssqo")
                    nc.scalar.activation(osq, op_, AF.Square, accum_out=ssq)
                    nrm = tpool.tile([C, 1], F32, tag="nrmo")
                    nc.scalar.activation(nrm, ssq, AF.Sqrt, bias=e6, scale=1.0 / Dh)
                    rms = tpool.tile([C, 1], F32, tag="rmso")
                    nc.vector.reciprocal(rms, nrm)
                    nc.scalar.activation(onorm[:, h * Dh:(h + 1) * Dh], op_,
                                         AF.Copy, scale=rms)

                    # --- state update ---
                    dsp = psm.tile([128, Dh], F32, tag="psmall")
                    nc.tensor.matmul(dsp[po:po + Dh, :],
                                     sb_k16[ci][:, h * Dh:(h + 1) * Dh],
                                     xv, start=True, stop=True,
                                     tile_position=(0, po))
                    nc.vector.tensor_add(sb_S32[p][po:po + Dh, :],
                                         sb_S32[p][po:po + Dh, :],
                                         dsp[po:po + Dh, :])
                    nc.vector.tensor_copy(S16h, sb_S32[p][po:po + Dh, :])

                # place my heads in my 256-col slot (others zeroed via host
                # 0/1 slotmask) so a bf16 ReduceScatter assembles o_full
                onM = tpool.tile([128, D], BF16, tag="onM")
                for sl in range(4):
                    nc.scalar.activation(onM[:, 256 * sl:256 * (sl + 1)], onorm,
                                         AF.Copy, scale=sb_smask[:, sl:sl + 1])
                rp = 512 * (ci % 4) + 128 * (ci // 4)
                nc.sync.dma_start(out=s_pin[rp:rp + 128, :], in_=onM)

        # ---- phase 5: masked-slot ReduceScatter (= head AllToAll) + out-proj + LN ----
        # four quarter-RS calls: block j holds sub-chunk j of every
        # core's slice, so early blocks overlap the scan tail and each
        # core's quarters land in its own 512-row slice in order
        for j in range(4):
            nc.gpsimd.collective_compute(
                "ReduceScatter", ALU.add,
                replica_groups=[[0, 1, 2, 3], [4, 5, 6, 7]],
                ins=[s_pin[512 * j:512 * (j + 1), :]],
                outs=[s_prs[128 * j:128 * (j + 1), :]],
            )

        sb_oT = []
        for dk in range(8):
            t = persist.tile([128, 512], BF16, tag=f"oT{dk}")
            nc.sync.dma_start_transpose(
                out=t, in_=s_prs[:, 128 * dk:128 * (dk + 1)])
            sb_oT.append(t)

        with tc.tile_pool(name="fin", bufs=2) as fin, \
             tc.tile_pool(name="py1", bufs=2, space="PSUM") as py1, \
             tc.tile_pool(name="py2", bufs=2, space="PSUM") as py2:
            for tt in range(4):
                tsl = slice(tt * 128, (tt + 1) * 128)
                yp1 = py1.tile([128, 512], F32, tag="yp1")
                yp2 = py2.tile([128, 512], F32, tag="yp2")
                for dk in range(8):
                    lhs = sb_oT[dk][:, tsl]
                    nc.tensor.matmul(yp1, lhs, sb_wo[dk][:, 0:512],
                                     start=(dk == 0), stop=(dk == 7))
                    nc.tensor.matmul(yp2, lhs, sb_wo[dk][:, 512:1024],
                                     start=(dk == 0), stop=(dk == 7))
                xr = fin.tile([128, D], F32, tag="xr")
                nc.sync.dma_start(out=xr, in_=x_res[tsl, :])
                y = fin.tile([128, D], F32, tag="ybuf")
                nc.vector.tensor_add(y[:, 0:512], yp1, xr[:, 0:512])
                nc.vector.tensor_add(y[:, 512:1024], yp2, xr[:, 512:1024])

                stats = fin.tile([128, 2, 6], F32, tag="stats")
                nc.vector.bn_stats(stats[:, 0, :], y[:, 0:512])
                nc.vector.bn_stats(stats[:, 1, :], y[:, 512:1024])
                mv = fin.tile([128, 2], F32, tag="mv")
                nc.vector.bn_aggr(mv, stats)
                sd = fin.tile([128, 1], F32, tag="sd")
                nc.scalar.activation(sd, mv[:, 1:2], AF.Sqrt, bias=e5)
                rsd = fin.tile([128, 1], F32, tag="rsd")
                nc.vector.reciprocal(rsd, sd)
                yn = fin.tile([128, D], F32, tag="yn")
                nc.vector.tensor_scalar(yn, y, mv[:, 0:1], rsd,
                                        op0=ALU.subtract, op1=ALU.mult)
                yf = fin.tile([128, D], F32, tag="yf")
                nc.vector.tensor_mul(yf, yn, sb_lngM)
                nc.vector.tensor_add(yf, yf, sb_lnbM)
                nc.sync.dma_start(out=out[tsl, :], in_=yf)

    nc.compile()
    return nc


def _host_inputs(inputs):
    x = np.ascontiguousarray(np.asarray(inputs["x"], np.float32))
    Wq = np.asarray(inputs["Wq"], np.float32)
    Wk = np.asarray(inputs["Wk"], np.float32)
    Wv = np.asarray(inputs["Wv"], np.float32)
    Wb = np.asarray(inputs["Wb"], np.float32)
    o_gamma = np.asarray(inputs["o_gamma"], np.float32)
    Wo = np.ascontiguousarray(np.asarray(inputs["Wo"], np.float32))
    Wo = np.ascontiguousarray(
        np.asarray(inputs["o_gamma"], np.float32).reshape(-1)[:, None] * Wo)
    ln_g = np.asarray(inputs["ln_g"], np.float32)
    ln_b = np.asarray(inputs["ln_b"], np.float32)

    idx = np.arange(C)
    same = (idx[:, None] // 32) == (idx[None, :] // 32)
    sl = idx[:, None] > idx[None, :]
    su = idx[:, None] < idx[None, :]
    m_bdsl = np.where(same & sl, 1.0, 0.0).astype(np.float16)
    m_bdsu = np.where(same & su, 1.0, 0.0).astype(np.float16)
    m_su = np.where(su, 1.0, 0.0).astype(np.float16)
    m_tiu = np.where(idx[:, None] <= idx[None, :], 1.0, 0.0).astype(np.float16)
    ident = np.eye(C, dtype=np.float16)

    in_maps = []
    for c in range(NCORE):
        b, g = c // 4, c % 4
        hs = NCH * g
        cols = slice(hs * Dh, (hs + NCH) * Dh)
        wqkv_c = np.ascontiguousarray(np.concatenate(
            [Wq[:, cols], Wk[:, cols], Wv[:, cols], Wb[:, hs:hs + NCH]], axis=1))
        smask = np.zeros((1, 4), np.float32)
        smask[0, g] = 1.0
        in_maps.append({
            "x_b": x[b],
            "x_res": np.ascontiguousarray(x[b, 512 * g:512 * (g + 1)]),
            "wqkv": wqkv_c,
            "wo": Wo,
            "slotmask": smask,
            "ogam": np.ascontiguousarray(o_gamma[hs:hs + NCH]),
            "lng": np.ascontiguousarray(ln_g[None, :]),
            "lnb": np.ascontiguousarray(ln_b[None, :]),
            "m_bdsl": m_bdsl, "m_bdsu": m_bdsu, "m_su": m_su,
            "m_tiu": m_tiu, "ident": ident,
        })
    return in_maps


_NC_CACHE = {}


def _get_program():
    if "nc" not in _NC_CACHE:
        _NC_CACHE["nc"] = _build_program()
    return _NC_CACHE["nc"]


def _install_ntff_hook():
    import sys, types
    try:
        import antenv.axon_hooks  # noqa: F401
        return
    except ImportError:
        pass
    from trn_agent_boot.trn_boot import _ntff_profile_via_ctypes
    hook = _ntff_profile_via_ctypes('/opt/axon/libaxon_pjrt.so')
    mod = types.ModuleType('antenv.axon_hooks')
    mod.get_axon_ntff_profile_hook = lambda: hook
    mod.set_axon_ntff_profile_hook = lambda h: None
    sys.modules['antenv.axon_hooks'] = mod


def kernel(trace=False, **inputs):
    from concourse.bass_utils import run_bass_kernel_spmd
    if trace:
        _install_ntff_hook()
    nc = _get_program()
    in_maps = _host_inputs(inputs)
    res = run_bass_kernel_spmd(nc, in_maps, list(range(NCORE)), trace=trace)
    out = np.zeros((B, T, D), np.float32)
    for c in range(NCORE):
        b, g = c // 4, c % 4
        out[b, 512 * g:512 * (g + 1)] = res.results[c]["out"]
    if trace:
        return out, res
    return out



# revision 47
# speedup vs baseline: 1.1103x; 1.1103x over previous
"""Trainium2 Bass kernel for nn_Block_58609123721506 (DeltaNet-style block).

Sharding: 8 cores = 2 batches x 4 head-groups (4 heads each).
Per core: bf16 qkv/beta projections, fp16 chunked delta-rule scan
(C=128 chunks, blocked-32 forward substitution with 3-stage Neumann
doubling for the diagonal 32x32 inverses), per-head RMSNorm, AllToAll
to token-shard the output projection, bf16 out-projection, fp32
residual + LayerNorm. Each core returns a disjoint (512, 1024) slice.
"""
import numpy as np

B, T, D, H = 2, 2048, 1024, 16
Dh = 64          # head dim
C = 128          # chunk length
NCH = 4          # heads per core
NCHUNK = T // C  # 16
NCORE = 8
WQKV_COLS = 3 * NCH * Dh + NCH  # 772
SCR_COLS = 3 * NCH * Dh        # 768 : q(256) k(256) kbneg(256)


def _build_program():
    from contextlib import ExitStack
    import concourse.bass as bass
    import concourse.mybir as mybir
    from concourse import bacc
    from concourse.tile import TileContext

    dt = mybir.dt
    F32, F16, BF16 = dt.float32, dt.float16, dt.bfloat16
    AF = mybir.ActivationFunctionType
    ALU = mybir.AluOpType

    nc = bacc.Bacc(None, target_bir_lowering=False, debug=True)

    x_b = nc.declare_dram_parameter("x_b", [T, D], F32, isOutput=False)
    x_res = nc.declare_dram_parameter("x_res", [T // 4, D], F32, isOutput=False)
    wqkv = nc.declare_dram_parameter("wqkv", [D, WQKV_COLS], F32, isOutput=False)
    wo = nc.declare_dram_parameter("wo", [D, D], F32, isOutput=False)
    ogam = nc.declare_dram_parameter("ogam", [NCH, Dh], F32, isOutput=False)
    lng = nc.declare_dram_parameter("lng", [1, D], F32, isOutput=False)
    lnb = nc.declare_dram_parameter("lnb", [1, D], F32, isOutput=False)
    slotmask = nc.declare_dram_parameter("slotmask", [1, 4], F32, isOutput=False)
    m_bdsl = nc.declare_dram_parameter("m_bdsl", [C, C], F16, isOutput=False)
    m_bdsu = nc.declare_dram_parameter("m_bdsu", [C, C], F16, isOutput=False)
    m_su = nc.declare_dram_parameter("m_su", [C, C], F16, isOutput=False)
    m_tiu = nc.declare_dram_parameter("m_tiu", [C, C], F16, isOutput=False)
    ident = nc.declare_dram_parameter("ident", [C, C], F16, isOutput=False)
    out = nc.declare_dram_parameter("out", [T // 4, D], F32, isOutput=True)

    s_xb = nc.dram_tensor("s_xb", [T, D], BF16)
    s_qkb = nc.dram_tensor("s_qkb", [T, SCR_COLS], F16)
    s_pin = nc.dram_tensor("s_pin", [T, D], BF16)
    s_prs = nc.dram_tensor("s_prs", [T // 4, D], BF16)

    def bcast_from_dram(ap_row, parts):
        """AP reading one DRAM row broadcast over `parts` partitions."""
        return bass.AP(tensor=ap_row.tensor, offset=ap_row.offset,
                       ap=[[0, parts]] + ap_row.ap[1:])

    with TileContext(nc) as tc, ExitStack() as ctx:
        const = ctx.enter_context(tc.tile_pool(name="const", bufs=1))
        persist = ctx.enter_context(tc.tile_pool(name="persist", bufs=1))

        # ---- constants ----
        sb_masks = {}
        for name, ap in (("m_bdsl", m_bdsl), ("m_bdsu", m_bdsu),
                         ("m_su", m_su), ("m_tiu", m_tiu), ("ident", ident)):
            t = const.tile([C, C], F16, tag=name)
            nc.sync.dma_start(out=t, in_=ap[:, :])
            sb_masks[name] = t
        # broadcast-materialized row vectors
        sb_gamM = const.tile([128, NCH * Dh], F32, tag="gamM")
        for h in range(NCH):
            nc.gpsimd.dma_start(out=sb_gamM[:, h * Dh:(h + 1) * Dh],
                                in_=bcast_from_dram(ogam[h:h + 1, :], 128))
        sb_lngM = const.tile([128, D], F32, tag="lngM")
        nc.gpsimd.dma_start(out=sb_lngM, in_=bcast_from_dram(lng[0:1, :], 128))
        sb_lnbM = const.tile([128, D], F32, tag="lnbM")
        nc.gpsimd.dma_start(out=sb_lnbM, in_=bcast_from_dram(lnb[0:1, :], 128))
        sb_smask = const.tile([128, 4], F32, tag="smask")
        nc.gpsimd.dma_start(out=sb_smask, in_=bcast_from_dram(slotmask[0:1, :], 128))
        e6 = const.tile([128, 1], F32, tag="e6")
        nc.vector.memset(e6, 1e-6)
        e5 = const.tile([128, 1], F32, tag="e5")
        nc.vector.memset(e5, 1e-5)

        # ---- weights: load fp32, cast to bf16 ----
        sb_wqkv = []
        sb_wo = []
        with tc.tile_pool(name="wstage", bufs=2) as wst:
            for dk in range(8):
                wtmp = wst.tile([128, WQKV_COLS], F32, tag="wstage")
                nc.sync.dma_start(out=wtmp, in_=wqkv[dk * 128:(dk + 1) * 128, :])
                wb = const.tile([128, WQKV_COLS], BF16, tag=f"wqkv{dk}")
                nc.scalar.copy(out=wb, in_=wtmp)
                sb_wqkv.append(wb)
            for dk in range(8):
                wtmp = wst.tile([128, D], F32, tag="wstage2")
                nc.sync.dma_start(out=wtmp, in_=wo[dk * 128:(dk + 1) * 128, :])
                wb = const.tile([128, D], BF16, tag=f"wo{dk}")
                nc.scalar.copy(out=wb, in_=wtmp)
                sb_wo.append(wb)

        sb_k16 = []   # token-layout k fp16 (128, 256) per ti
        sb_vb = []    # beta*v fp16 (128, 256) per ti
        sb_bcol = []  # beta fp32 (128, 4) per ti

        with tc.tile_pool(name="xbt", bufs=1) as xbt_pool:
            # ---- phase 1: x -> bf16 -> DRAM -> transposed tiles ----
            with tc.tile_pool(name="xstage", bufs=3) as xst:
                for ti in range(16):
                    xt = xst.tile([128, D], F32, tag="xload")
                    nc.sync.dma_start(out=xt, in_=x_b[ti * 128:(ti + 1) * 128, :])
                    xc = xst.tile([128, D], BF16, tag="xcast")
                    nc.scalar.copy(out=xc, in_=xt)
                    nc.sync.dma_start(out=s_xb[ti * 128:(ti + 1) * 128, :], in_=xc)
            sb_xbT = []
            for dk in range(8):
                t = xbt_pool.tile([128, T], BF16, tag=f"xbT{dk}")
                nc.sync.dma_start_transpose(out=t, in_=s_xb[:, dk * 128:(dk + 1) * 128])
                sb_xbT.append(t)

            # ---- phase 2: projections (two-pass, sqrt batched once) ----
            sb_act = []    # silu(q|k) f32 per ti
            sb_nb = []     # -beta per ti
            ssqA = persist.tile([128, 16, 8], F32, tag="ssqA")
            with tc.tile_pool(name="proj", bufs=2) as pst, \
                 tc.tile_pool(name="pp1", bufs=2, space="PSUM") as pp1, \
                 tc.tile_pool(name="pp2", bufs=2, space="PSUM") as pp2:
                for ti in range(16):
                    ts_ = slice(ti * 128, (ti + 1) * 128)
                    ps1 = pp1.tile([128, 512], F32, tag="ps1")
                    ps2 = pp2.tile([128, 260], F32, tag="ps2")
                    for dk in range(8):
                        lhs = sb_xbT[dk][:, ts_]
                        nc.tensor.matmul(ps1, lhs, sb_wqkv[dk][:, 0:512],
                                         start=(dk == 0), stop=(dk == 7))
                        nc.tensor.matmul(ps2, lhs, sb_wqkv[dk][:, 512:772],
                                         start=(dk == 0), stop=(dk == 7))
                    # beta = 2*sigmoid(x) = 1 + tanh(x/2); avoids a Sigmoid
                    # table load (Tanh shares the Silu act table)
                    tb = pst.tile([128, NCH], F32, tag="tb")
                    nc.scalar.activation(tb, ps2[:, 256:260], AF.Tanh, scale=0.5)
                    bcol = persist.tile([128, NCH], F32, tag=f"bcol{ti}")
                    nc.vector.tensor_scalar_add(bcol, tb, 1.0)
                    nbcol = persist.tile([128, NCH], F32, tag=f"nbcol{ti}")
                    nc.vector.tensor_scalar(nbcol, tb, 1.0, -1.0,
                                            op0=ALU.add, op1=ALU.mult)
                    sb_bcol.append(bcol)
                    sb_nb.append(nbcol)

                    vb = persist.tile([128, 256], F16, tag=f"vb{ti}")
                    act = persist.tile([128, 512], F32, tag=f"act{ti}")
                    nc.scalar.activation(act, ps1, AF.Silu)
                    sb_act.append(act)
                    sq = pst.tile([128, 512], F32, tag="sq")
                    nc.vector.tensor_mul(sq, act, act)
                    nc.vector.tensor_reduce(
                        ssqA[:, ti, :], bass.AP(tensor=sq.tensor, offset=sq.offset,
                                                ap=sq.ap[:1] + [[64, 8], [1, 64]]),
                        axis=mybir.AxisListType.X, op=ALU.add)
                    for h in range(NCH):
                        hs = slice(h * Dh, (h + 1) * Dh)
                        nc.vector.tensor_scalar_mul(vb[:, hs], ps2[:, hs],
                                                    bcol[:, h:h + 1])
                    sb_vb.append(vb)

                # one table switch (Silu -> Sqrt) for the whole kernel
                nrmA = persist.tile([128, 16, 8], F32, tag="nrmA")
                nc.scalar.activation(nrmA[:, :, :], ssqA[:, :, :], AF.Sqrt, bias=e6)
                rnA = persist.tile([128, 16, 8], F32, tag="rnA")
                nc.vector.reciprocal(rnA[:, :, :], nrmA[:, :, :])

                for ti in range(16):
                    ts_ = slice(ti * 128, (ti + 1) * 128)
                    act = sb_act[ti]
                    k16 = persist.tile([128, 256], F16, tag=f"k16_{ti}")
                    q16 = pst.tile([128, 256], F16, tag="q16")
                    kbn = pst.tile([128, 256], F16, tag="kbn")
                    for h in range(NCH):
                        hs = slice(h * Dh, (h + 1) * Dh)
                        ks_ = slice(256 + h * Dh, 256 + (h + 1) * Dh)
                        nc.vector.tensor_scalar_mul(q16[:, hs], act[:, hs],
                                                    rnA[:, ti, h:h + 1])
                        nc.vector.tensor_scalar_mul(k16[:, hs], act[:, ks_],
                                                    rnA[:, ti, 4 + h:5 + h])
                        nc.vector.tensor_scalar(kbn[:, hs], act[:, ks_],
                                                rnA[:, ti, 4 + h:5 + h],
                                                sb_nb[ti][:, h:h + 1],
                                                op0=ALU.mult, op1=ALU.mult)
                    sb_k16.append(k16)

                    nc.sync.dma_start(out=s_qkb[ts_, 0:256], in_=q16)
                    nc.sync.dma_start(out=s_qkb[ts_, 256:512], in_=k16)
                    nc.sync.dma_start(out=s_qkb[ts_, 512:768], in_=kbn)

        # ---- phase 3+4: chunked delta-rule scan ----
        sb_S32 = []
        sb_S16 = []
        for p in range(2):  # head pairs (0,1) and (2,3)
            s32 = persist.tile([128, Dh], F32, tag=f"S32_{p}")
            s16 = persist.tile([128, Dh], F16, tag=f"S16_{p}")
            nc.vector.memset(s32, 0.0)
            nc.vector.memset(s16, 0.0)
            sb_S32.append(s32)
            sb_S16.append(s16)

        MB_SL = sb_masks["m_bdsl"]
        MB_SU = sb_masks["m_bdsu"]
        M_SU = sb_masks["m_su"]
        M_TIU = sb_masks["m_tiu"]
        IDT = sb_masks["ident"]

        with tc.tile_pool(name="tpool", bufs=4) as tpool, \
             tc.tile_pool(name="dpool", bufs=4) as dpool, \
             tc.tile_pool(name="pg", bufs=2, space="PSUM") as pg, \
             tc.tile_pool(name="pd", bufs=2, space="PSUM") as pd, \
             tc.tile_pool(name="psm", bufs=4, space="PSUM") as psm:
            for ci in range(NCHUNK):
                rs = slice(ci * C, (ci + 1) * C)
                bqT = [tpool.tile([128, 2 * C], F16, tag=f"bqT{p}", name=f"bqT{p}")
                       for p in range(2)]
                kT = [tpool.tile([128, C], F16, tag=f"kT{p}", name=f"kT{p}")
                      for p in range(2)]
                for p in range(2):
                    co = p * 128
                    nc.sync.dma_start_transpose(out=bqT[p][:, 0:C],
                                                in_=s_qkb[rs, 512 + co:512 + co + 128])
                    nc.sync.dma_start_transpose(out=bqT[p][:, C:2 * C],
                                                in_=s_qkb[rs, co:co + 128])
                    nc.sync.dma_start_transpose(out=kT[p],
                                                in_=s_qkb[rs, 256 + co:256 + co + 128])

                onorm = tpool.tile([128, NCH * Dh], BF16, tag="onorm")

                for h in range(NCH):
                    p, sub = h // 2, h % 2
                    po = sub * Dh
                    kTh = kT[p][po:po + Dh, :]
                    qTh = bqT[p][po:po + Dh, C:2 * C]
                    bTh = bqT[p][po:po + Dh, 0:C]
                    S16h = sb_S16[p][po:po + Dh, :]

                    # --- chunk-local prep ---
                    # gneg[i,j] = -beta_j k_i.k_j ; gneg2[i,j] = -beta_i k_i.k_j
                    gall = pg.tile([C, 3 * C], F32, tag="g0")
                    nc.tensor.matmul(gall[:, 0:2 * C], kTh, bqT[p][po:po + Dh, :],
                                     start=True, stop=True, tile_position=(po, 0))
                    gneg = gall[:, 0:C]
                    kq = gall[:, C:2 * C]
                    gneg2 = gall[:, 2 * C:3 * C]
                    nc.tensor.matmul(gneg2, bTh, kTh, start=True, stop=True,
                                     tile_position=(po, 0))
                    sqk = dpool.tile([C, C], F16, tag="sqk")
                    nc.vector.tensor_mul(sqk, kq, M_TIU)
                    bdN = dpool.tile([C, C], F16, tag="bdN")
                    nc.vector.tensor_mul(bdN, gneg2, MB_SL)
                    ntil = dpool.tile([C, C], F16, tag="ntil")
                    nc.vector.tensor_mul(ntil, gneg, M_SU)
                    bdNt = dpool.tile([C, C], F16, tag="bdNt")
                    nc.vector.tensor_mul(bdNt, gneg, MB_SU)
                    pt = dpool.tile([C, C], F16, tag="pt")
                    nc.gpsimd.tensor_add(pt, IDT, bdNt)

                    # --- doubling: 3 stages ---
                    bm, bt = bdN, bdNt
                    for j in range(3):
                        bmp = pd.tile([C, C], F32, tag="pdb")
                        nc.tensor.matmul(bmp, bt, bm, start=True, stop=True)
                        bmn = dpool.tile([C, C], F16, tag=f"bm{j}")
                        nc.vector.tensor_copy(bmn, bmp)
                        if j < 2:
                            btp = pd.tile([C, C], F32, tag="pdb")
                            nc.tensor.matmul(btp, bm, bt, start=True, stop=True)
                            btn = dpool.tile([C, C], F16, tag=f"bt{j}")
                            nc.vector.tensor_copy(btn, btp)
                        else:
                            btn = None
                        ptp = pd.tile([C, C], F32, tag="pdb")
                        nc.tensor.matmul(ptp, bmn, pt, start=True, stop=True)
                        ptn = dpool.tile([C, C], F16, tag=f"pt{j}")
                        nc.vector.tensor_add(ptn, pt, ptp)
                        bm, bt, pt = bmn, btn, ptn

                    # --- sequential substitution ---
                    yp = psm.tile([C, Dh], F32, tag="psmall")
                    nc.tensor.matmul(yp, bTh, S16h, start=True, stop=True,
                                     tile_position=(po, 0))
                    rr = tpool.tile([C, Dh], F16, tag="rr")
                    nc.vector.tensor_add(rr, sb_vb[ci][:, h * Dh:(h + 1) * Dh], yp)

                    xv = tpool.tile([C, Dh], F16, tag="xv")
                    xp = psm.tile([C, Dh], F32, tag="psmall")
                    ac = psm.tile([C, Dh], F32, tag="psmall")
                    tst = tpool.tile([C, Dh], F16, tag="tst")
                    for i in range(4):
                        bs = slice(32 * i, 32 * i + 32)
                        if i == 0:
                            nc.tensor.matmul(xp[0:32, :], pt[0:32, 0:32], rr[0:32, :],
                                             start=True, stop=True,
                                             tile_position=(0, 0))
                        else:
                            nc.tensor.matmul(ac[bs, :], ntil[0:32 * i, bs],
                                             xv[0:32 * i, :], start=True, stop=True,
                                             tile_position=(0, 32 * i))
                            nc.vector.tensor_add(tst[bs, :], rr[bs, :], ac[bs, :])
                            nc.tensor.matmul(xp[bs, :], pt[bs, bs], tst[bs, :],
                                             start=True, stop=True,
                                             tile_position=(32 * i, 32 * i))
                        nc.scalar.copy(out=xv[bs, :], in_=xp[bs, :])

                    # --- output ---
                    op_ = psm.tile([C, Dh], F32, tag="psmall")
                    nc.tensor.matmul(op_, qTh, S16h, start=True, stop=False,
                                     tile_position=(po, 0))
                    nc.tensor.matmul(op_, sqk, xv, start=False, stop=True)
                    osq = tpool.tile([C, Dh], F32, tag="osq")
                    ssq = tpool.tile([C, 1], F32, tag="ssqo")
                    nc.scalar.activation(osq, op_, AF.Square, accum_out=ssq)
                    nrm = tpool.tile([C, 1], F32, tag="nrmo")
                    nc.scalar.activation(nrm, ssq, AF.Sqrt, bias=e6, scale=1.0 / Dh)
                    rms = tpool.tile([C, 1], F32, tag="rmso")
                    nc.vector.reciprocal(rms, nrm)
                    nc.scalar.activation(onorm[:, h * Dh:(h + 1) * Dh], op_,
                                         AF.Copy, scale=rms)

                    # --- state update ---
                    dsp = psm.tile([128, Dh], F32, tag="psmall")
                    nc.tensor.matmul(dsp[po:po + Dh, :],
                                     sb_k16[ci][:, h * Dh:(h + 1) * Dh],
                                     xv, start=True, stop=True,
                                     tile_position=(0, po))
                    nc.vector.tensor_add(sb_S32[p][po:po + Dh, :],
                                         sb_S32[p][po:po + Dh, :],
                                         dsp[po:po + Dh, :])
                    nc.scalar.copy(out=S16h, in_=sb_S32[p][po:po + Dh, :])

                # place my heads in my 256-col slot (others zeroed via host
                # 0/1 slotmask) so a bf16 ReduceScatter assembles o_full
                onM = tpool.tile([128, D], BF16, tag="onM")
                for sl in range(4):
                    nc.scalar.activation(onM[:, 256 * sl:256 * (sl + 1)], onorm,
                                         AF.Copy, scale=sb_smask[:, sl:sl + 1])
                nc.sync.dma_start(out=s_pin[rs, :], in_=onM)

        # ---- phase 5: masked-slot ReduceScatter (= head AllToAll) + out-proj + LN ----
        nc.gpsimd.collective_compute(
            "ReduceScatter", ALU.add,
            replica_groups=[[0, 1, 2, 3], [4, 5, 6, 7]],
            ins=[s_pin[:, :]],
            outs=[s_prs[:, :]],
        )

        sb_oT = []
        for dk in range(8):
            t = persist.tile([128, 512], BF16, tag=f"oT{dk}")
            nc.sync.dma_start_transpose(
                out=t, in_=s_prs[:, 128 * dk:128 * (dk + 1)])
            sb_oT.append(t)

        with tc.tile_pool(name="fin", bufs=2) as fin, \
             tc.tile_pool(name="py1", bufs=2, space="PSUM") as py1, \
             tc.tile_pool(name="py2", bufs=2, space="PSUM") as py2:
            for tt in range(4):
                tsl = slice(tt * 128, (tt + 1) * 128)
                yp1 = py1.tile([128, 512], F32, tag="yp1")
                yp2 = py2.tile([128, 512], F32, tag="yp2")
                for dk in range(8):
                    lhs = sb_oT[dk][:, tsl]
                    nc.tensor.matmul(yp1, lhs, sb_wo[dk][:, 0:512],
                                     start=(dk == 0), stop=(dk == 7))
                    nc.tensor.matmul(yp2, lhs, sb_wo[dk][:, 512:1024],
                                     start=(dk == 0), stop=(dk == 7))
                xr = fin.tile([128, D], F32, tag="xr")
                nc.sync.dma_start(out=xr, in_=x_res[tsl, :])
                y = fin.tile([128, D], F32, tag="ybuf")
                nc.vector.tensor_add(y[:, 0:512], yp1, xr[:, 0:512])
                nc.vector.tensor_add(y[:, 512:1024], yp2, xr[:, 512:1024])

                stats = fin.tile([128, 2, 6], F32, tag="stats")
                nc.vector.bn_stats(stats[:, 0, :], y[:, 0:512])
                nc.vector.bn_stats(stats[:, 1, :], y[:, 512:1024])
                mv = fin.tile([128, 2], F32, tag="mv")
                nc.vector.bn_aggr(mv, stats)
                sd = fin.tile([128, 1], F32, tag="sd")
                nc.scalar.activation(sd, mv[:, 1:2], AF.Sqrt, bias=e5)
                rsd = fin.tile([128, 1], F32, tag="rsd")
                nc.vector.reciprocal(rsd, sd)
                yn = fin.tile([128, D], F32, tag="yn")
                nc.vector.tensor_scalar(yn, y, mv[:, 0:1], rsd,
                                        op0=ALU.subtract, op1=ALU.mult)
                yf = fin.tile([128, D], F32, tag="yf")
                nc.vector.tensor_mul(yf, yn, sb_lngM)
                nc.vector.tensor_add(yf, yf, sb_lnbM)
                nc.sync.dma_start(out=out[tsl, :], in_=yf)

    nc.compile()
    return nc


def _host_inputs(inputs):
    x = np.ascontiguousarray(np.asarray(inputs["x"], np.float32))
    Wq = np.asarray(inputs["Wq"], np.float32)
    Wk = np.asarray(inputs["Wk"], np.float32)
    Wv = np.asarray(inputs["Wv"], np.float32)
    Wb = np.asarray(inputs["Wb"], np.float32)
    o_gamma = np.asarray(inputs["o_gamma"], np.float32)
    Wo = np.ascontiguousarray(np.asarray(inputs["Wo"], np.float32))
    Wo = np.ascontiguousarray(
        np.asarray(inputs["o_gamma"], np.float32).reshape(-1)[:, None] * Wo)
    ln_g = np.asarray(inputs["ln_g"], np.float32)
    ln_b = np.asarray(inputs["ln_b"], np.float32)

    idx = np.arange(C)
    same = (idx[:, None] // 32) == (idx[None, :] // 32)
    sl = idx[:, None] > idx[None, :]
    su = idx[:, None] < idx[None, :]
    m_bdsl = np.where(same & sl, 1.0, 0.0).astype(np.float16)
    m_bdsu = np.where(same & su, 1.0, 0.0).astype(np.float16)
    m_su = np.where(su, 1.0, 0.0).astype(np.float16)
    m_tiu = np.where(idx[:, None] <= idx[None, :], 1.0, 0.0).astype(np.float16)
    ident = np.eye(C, dtype=np.float16)

    in_maps = []
    for c in range(NCORE):
        b, g = c // 4, c % 4
        hs = NCH * g
        cols = slice(hs * Dh, (hs + NCH) * Dh)
        wqkv_c = np.ascontiguousarray(np.concatenate(
            [Wq[:, cols], Wk[:, cols], Wv[:, cols], Wb[:, hs:hs + NCH]], axis=1))
        smask = np.zeros((1, 4), np.float32)
        smask[0, g] = 1.0
        in_maps.append({
            "x_b": x[b],
            "x_res": np.ascontiguousarray(x[b, 512 * g:512 * (g + 1)]),
            "wqkv": wqkv_c,
            "wo": Wo,
            "slotmask": smask,
            "ogam": np.ascontiguousarray(o_gamma[hs:hs + NCH]),
            "lng": np.ascontiguousarray(ln_g[None, :]),
            "lnb": np.ascontiguousarray(ln_b[None, :]),
            "m_bdsl": m_bdsl, "m_bdsu": m_bdsu, "m_su": m_su,
            "m_tiu": m_tiu, "ident": ident,
        })
    return in_maps


_NC_CACHE = {}


def _get_program():
    if "nc" not in _NC_CACHE:
        _NC_CACHE["nc"] = _build_program()
    return _NC_CACHE["nc"]


def _install_ntff_hook():
    import sys, types
    try:
        import antenv.axon_hooks  # noqa: F401
        return
    except ImportError:
        pass
    from trn_agent_boot.trn_boot import _ntff_profile_via_ctypes
    hook = _ntff_profile_via_ctypes('/opt/axon/libaxon_pjrt.so')
    mod = types.ModuleType('antenv.axon_hooks')
    mod.get_axon_ntff_profile_hook = lambda: hook
    mod.set_axon_ntff_profile_hook = lambda h: None
    sys.modules['antenv.axon_hooks'] = mod


def kernel(trace=False, **inputs):
    from concourse.bass_utils import run_bass_kernel_spmd
    if trace:
        _install_ntff_hook()
    nc = _get_program()
    in_maps = _host_inputs(inputs)
    res = run_bass_kernel_spmd(nc, in_maps, list(range(NCORE)), trace=trace)
    out = np.zeros((B, T, D), np.float32)
    for c in range(NCORE):
        b, g = c // 4, c % 4
        out[b, 512 * g:512 * (g + 1)] = res.results[c]["out"]
    if trace:
        return out, res
    return out

